# revision 32
# baseline (speedup 1.0000x reference)
"""2D DCT-II (separable) kernel for Trainium2, data-parallel over 8 NeuronCores.

Problem: img [128, 1, 512, 512] f32 -> out [128, 1, 512, 512] f32 with
    out[b,0] = scale * (Cp @ img[b,0] @ Cq^T)
where Cp[p,m] = cos(pi*(2m+1)*p/1024), Cq[q,n] = cos(pi*(2n+1)*q/1024) and
scale[p,q] = (2/512)*row[p]*col[q] (1/sqrt2 on p==0 / q==0). Since M=N=512 the
two basis matrices are identical; the rank-1 scale is folded into them:
    C'[k,j] = s_k * cos(pi*(2j+1)*k/1024),  s_k = sqrt(2/512) * (1/sqrt2 if k==0 else 1)
    out[b] = C' @ img[b] @ C'^T

Per-core (16 images each): two PE matmul stages with the image/intermediate as
the stationary operand (both stages contract over the data's partition dim, so
no transposes are needed):
    stage1: Dt[n, p] = sum_m A[m, n] * C'T[m, p]   (lhsT = A tile, rhs = C'T)
    stage2: Y[p, q]  = sum_n Dt[n, p] * C'T[n, q]  (lhsT = Dt tile, rhs = C'T)
Matmuls run in float32r (TF32-like, ~11 mantissa bits) at full PE rate.

Stage 1 exploits the DCT-II reflection symmetry C'[p, 511-m] = (-1)^p C'[p, m]:
with E[m'] = A[m'] + A[511-m'] and O[m'] = A[m'] - A[511-m'] (m' < 256),
even output rows come from a 256-contraction with E, odd rows from O —
half the stage-1 PE cycles. The host passes the image as two row-halves
(bottom half row-reversed) so the fold pairs are partition-aligned; E/O are
formed on DVE/GpSimd; the even/odd output-row interleave happens inside the
PSUM->SBUF copies (stride-2 writes, same 1x copy cost).
"""

import sys
import numpy as np

for _p in ("/opt/trn_rl_repo", "/opt/pypackages"):
    if _p not in sys.path:
        sys.path.append(_p)

import concourse.tile as tile  # noqa: E402
from concourse import bacc, mybir  # noqa: E402
from concourse.bass_utils import run_bass_kernel_spmd  # noqa: E402

N_CORES = 8
B_FULL = 128
S = 512  # image side
H = S // 2
BPC = B_FULL // N_CORES  # images per core
T = S // 128  # 4 partition tiles per image side


def _basis_f32() -> np.ndarray:
    """C'T[j, k] = s_k * cos(pi*(2j+1)*k/1024), shape [512, 512] f32."""
    j = np.arange(S, dtype=np.float64)
    k = np.arange(S, dtype=np.float64)
    c = np.cos(np.pi * (2.0 * j[:, None] + 1.0) * k[None, :] / (2.0 * S))
    s = np.full(S, np.sqrt(2.0 / S), dtype=np.float64)
    s[0] /= np.sqrt(2.0)
    return (c * s[None, :]).astype(np.float32)


def _build():
    nc = bacc.Bacc("TRN2", target_bir_lowering=False, debug=False)
    # Image passed as two halves: rows 0:256 forward, rows 511:255 reversed
    # (host-side flip) so fold pairs m' <-> 511-m' are partition-aligned with
    # plain positive-stride DMAs.
    imga_d = nc.dram_tensor(
        "imga", [BPC, H, S], mybir.dt.float32r, kind="ExternalInput"
    ).ap()
    imgb_d = nc.dram_tensor(
        "imgb", [BPC, H, S], mybir.dt.float32r, kind="ExternalInput"
    ).ap()
    ct_d = nc.dram_tensor("ct", [S, S], mybir.dt.float32r, kind="ExternalInput").ap()
    ce_d = nc.dram_tensor("ce", [H, H], mybir.dt.float32r, kind="ExternalInput").ap()
    co_d = nc.dram_tensor("co", [H, H], mybir.dt.float32r, kind="ExternalInput").ap()
    out_d = nc.dram_tensor("out", [BPC, S, S], mybir.dt.float32, kind="ExternalOutput").ap()

    out_v = out_d.rearrange("b (t p) q -> b p t q", p=128)
    imga_v = imga_d.rearrange("b (t p) n -> b p t n", p=128)
    imgb_v = imgb_d.rearrange("b (t p) n -> b p t n", p=128)
    ct_v = ct_d.rearrange("(t p) k -> t p k", p=128)
    ce_v = ce_d.rearrange("(t p) k -> t p k", p=128)
    co_v = co_d.rearrange("(t p) k -> t p k", p=128)

    with tile.TileContext(nc) as tc:
        with (
            tc.tile_pool(name="const", bufs=1) as cpool,
            tc.tile_pool(name="a", bufs=10) as apool,
            tc.tile_pool(name="a01", bufs=8) as a01pool,
            tc.tile_pool(name="eo", bufs=16) as eopool,
            tc.tile_pool(name="dt", bufs=2) as dtpool,
            tc.tile_pool(name="o", bufs=8) as opool,
            tc.tile_pool(name="ps1", bufs=4, space="PSUM") as ps1pool,
            tc.tile_pool(name="ps2", bufs=4, space="PSUM") as ps2pool,
        ):
            # ce tile 0 first — the very first matmul needs only it plus
            # image 0's two halves; the remaining constants follow them.
            ce_sb = cpool.tile([128, 2, H], mybir.dt.float32r)
            co_sb = cpool.tile([128, 2, H], mybir.dt.float32r)
            ct_sb = cpool.tile([128, T, S], mybir.dt.float32r)
            nc.sync.dma_start(co_sb[:, 0, :], co_v[0])

            for i in range(BPC):
                # Forward half [m' tile, n] and reversed half: ar[:, t, :]
                # holds rows 511-... so partition j of (af[:,t], ar[:,t]) is
                # the fold pair (m', 511-m').
                if i < 2:
                    # Per-half loads in separate tiles: the first fold (and so
                    # the first matmul) starts after 512 KB instead of 1 MB.
                    ah = []
                    for mh, src in (
                        (0, imga_v[i, :, 0, :]),
                        (1, imgb_v[i, :, 0, :]),
                        (2, imga_v[i, :, 1, :]),
                        (3, imgb_v[i, :, 1, :]),
                    ):
                        t = a01pool.tile(
                            [128, S], mybir.dt.float32r, tag="a01", name=f"ah_{i}_{mh}"
                        )
                        nc.sync.dma_start(t[:], src)
                        ah.append(t)
                    af0, ar0, af1, ar1 = ah[0][:, :], ah[1][:, :], ah[2][:, :], ah[3][:, :]
                else:
                    af = apool.tile([128, 2, S], mybir.dt.float32r, tag="a", name=f"af_{i}")
                    ar = apool.tile([128, 2, S], mybir.dt.float32r, tag="a", name=f"ar_{i}")
                    nc.sync.dma_start(af[:], imga_v[i])
                    nc.sync.dma_start(ar[:], imgb_v[i])
                    af0, ar0, af1, ar1 = af[:, 0, :], ar[:, 0, :], af[:, 1, :], ar[:, 1, :]
                if i == 0:
                    # Remaining constants, ordered by first use.
                    nc.sync.dma_start(co_sb[:, 1, :], co_v[1])
                    for t in range(2):
                        nc.sync.dma_start(ce_sb[:, t, :], ce_v[t])
                    for t in range(T):
                        nc.sync.dma_start(ct_sb[:, t, :], ct_v[t])

                # Fold: E = A + rev(A), O = A - rev(A)  (each [256, 512], 2 tiles)
                e0 = eopool.tile([128, S], mybir.dt.float32r, tag="eo", name=f"e0_{i}")
                e1 = eopool.tile([128, S], mybir.dt.float32r, tag="eo", name=f"e1_{i}")
                o0 = eopool.tile([128, S], mybir.dt.float32r, tag="eo", name=f"o0_{i}")
                o1 = eopool.tile([128, S], mybir.dt.float32r, tag="eo", name=f"o1_{i}")
                nc.gpsimd.tensor_sub(o0[:], af0, ar0)
                nc.gpsimd.tensor_sub(o1[:], af1, ar1)
                nc.vector.tensor_add(e0[:], af0, ar0)
                nc.vector.tensor_add(e1[:], af1, ar1)
                e_t, o_t = (e0, e1), (o0, o1)

                # stage 1 (folded): Dt[n, 2k] from E/ce, Dt[n, 2k+1] from O/co.
                # ps1[nt] cols 0:256 hold even-p, cols 256:512 odd-p.
                ps1 = [ps1pool.tile([128, S], mybir.dt.float32, tag="ps1", name=f"ps1_{i}_{j}") for j in range(T)]
                # O-part first: the gpsimd subs are ready earlier than the DVE
                # adds (which queue behind the previous image's mid copies).
                for nt in range(T):
                    nts = slice(nt * 128, (nt + 1) * 128)
                    for mh in range(2):
                        nc.tensor.matmul(
                            ps1[nt][:, H:S],
                            o_t[mh][:, nts],
                            co_sb[:, mh, :],
                            start=(mh == 0),
                            stop=(mh == 1),
                        )
                    for mh in range(2):
                        nc.tensor.matmul(
                            ps1[nt][:, 0:H],
                            e_t[mh][:, nts],
                            ce_sb[:, mh, :],
                            start=(mh == 0),
                            stop=(mh == 1),
                        )
                dt_sb = dtpool.tile([128, T, S], mybir.dt.float32r, tag="dt")
                for nt in range(T):
                    # One mid-copy pair on ACT to keep DVE under the PE span.
                    eng = nc.scalar.copy if nt == 3 else nc.vector.tensor_copy
                    eng(dt_sb[:, nt, 0:S:2], ps1[nt][:, 0:H])
                    eng(dt_sb[:, nt, 1:S:2], ps1[nt][:, H:S])

                # stage 2 (p-outer): Y[p, q] = sum_n Dt[n, p] C'T[n, q]
                # Output staged in 2-tile chunks: fewer DMA descriptors while
                # keeping the drain pipelined.
                last = i == BPC - 1
                for ph in range(2):
                    o_sb = opool.tile(
                        [128, 2, S], mybir.dt.float32, tag="o", name=f"o_{i}_{ph}"
                    )
                    for pj in range(2):
                        pt = ph * 2 + pj
                        ps2 = ps2pool.tile(
                            [128, S], mybir.dt.float32, tag="ps2", name=f"ps2_{i}_{pt}"
                        )
                        for nt in range(T):
                            nc.tensor.matmul(
                                ps2[:],
                                dt_sb[:, nt, pt * 128 : (pt + 1) * 128],
                                ct_sb[:, nt, :],
                                start=(nt == 0),
                                stop=(nt == T - 1),
                            )
                        nc.scalar.copy(o_sb[:, pj, :], ps2[:])
                        if last:
                            # Drain the final image per p-tile on alternating
                            # queues so the tail DMA overlaps the last matmuls.
                            eng = nc.scalar if pt % 2 == 0 else nc.sync
                            eng.dma_start(out_v[i, :, pt, :], o_sb[:, pj, :])
                    if not last:
                        if ph == 0:
                            nc.scalar.dma_start(out_v[i, :, 0:2, :], o_sb[:])
                        else:
                            nc.sync.dma_start(out_v[i, :, 2:4, :], o_sb[:])
    nc.compile()
    return nc


_NC_CACHE = None


def _get_nc():
    global _NC_CACHE
    if _NC_CACHE is None:
        _NC_CACHE = _build()
    return _NC_CACHE


def run_sharded(img: np.ndarray, **spmd_kwargs):
    """img [128, 1, 512, 512] f32 -> (out [128, 1, 512, 512] f32, BassKernelResults)."""
    img = np.ascontiguousarray(np.asarray(img, dtype=np.float32)).reshape(B_FULL, S, S)
    imga = np.ascontiguousarray(img[:, :H, :])
    imgb = np.ascontiguousarray(img[:, :H - 1 :-1, :])  # rows 511..256 reversed
    ct = _basis_f32()
    ce = np.ascontiguousarray(ct[:H, 0::2])
    co = np.ascontiguousarray(ct[:H, 1::2])
    nc = _get_nc()
    in_maps = [
        {
            "imga": imga[k * BPC : (k + 1) * BPC],
            "imgb": imgb[k * BPC : (k + 1) * BPC],
            "ct": ct,
            "ce": ce,
            "co": co,
        }
        for k in range(N_CORES)
    ]
    res = run_bass_kernel_spmd(nc, in_maps, core_ids=list(range(N_CORES)), **spmd_kwargs)
    out = np.empty((B_FULL, S, S), dtype=np.float32)
    for k in range(N_CORES):
        out[k * BPC : (k + 1) * BPC] = res.results[k]["out"]
    return out.reshape(B_FULL, 1, S, S), res


def kernel(img: np.ndarray) -> np.ndarray:
    out, _ = run_sharded(img)
    return out


# revision 34
# speedup vs baseline: 1.0382x; 1.0382x over previous
"""2D DCT-II (separable) kernel for Trainium2, data-parallel over 8 NeuronCores.

Problem: img [128, 1, 512, 512] f32 -> out [128, 1, 512, 512] f32 with
    out[b,0] = scale * (Cp @ img[b,0] @ Cq^T)
where Cp[p,m] = cos(pi*(2m+1)*p/1024), Cq[q,n] = cos(pi*(2n+1)*q/1024) and
scale[p,q] = (2/512)*row[p]*col[q] (1/sqrt2 on p==0 / q==0). Since M=N=512 the
two basis matrices are identical; the rank-1 scale is folded into them:
    C'[k,j] = s_k * cos(pi*(2j+1)*k/1024),  s_k = sqrt(2/512) * (1/sqrt2 if k==0 else 1)
    out[b] = C' @ img[b] @ C'^T

Per-core (16 images each): two PE matmul stages with the image/intermediate as
the stationary operand (both stages contract over the data's partition dim, so
no transposes are needed):
    stage1: Dt[n, p] = sum_m A[m, n] * C'T[m, p]   (lhsT = A tile, rhs = C'T)
    stage2: Y[p, q]  = sum_n Dt[n, p] * C'T[n, q]  (lhsT = Dt tile, rhs = C'T)
Matmuls run in float32r (TF32-like, ~11 mantissa bits) at full PE rate.

Stage 1 exploits the DCT-II reflection symmetry C'[p, 511-m] = (-1)^p C'[p, m]:
with E[m'] = A[m'] + A[511-m'] and O[m'] = A[m'] - A[511-m'] (m' < 256),
even output rows come from a 256-contraction with E, odd rows from O —
half the stage-1 PE cycles. The host passes the image as two row-halves
(bottom half row-reversed) so the fold pairs are partition-aligned; E/O are
formed on DVE/GpSimd; the even/odd output-row interleave happens inside the
PSUM->SBUF copies (stride-2 writes, same 1x copy cost).
"""

import sys
import numpy as np

for _p in ("/opt/trn_rl_repo", "/opt/pypackages"):
    if _p not in sys.path:
        sys.path.append(_p)

import concourse.tile as tile  # noqa: E402
from concourse import bacc, mybir  # noqa: E402
from concourse.bass_utils import run_bass_kernel_spmd  # noqa: E402

N_CORES = 8
B_FULL = 128
S = 512  # image side
H = S // 2
BPC = B_FULL // N_CORES  # images per core
T = S // 128  # 4 partition tiles per image side


def _basis_f32() -> np.ndarray:
    """C'T[j, k] = s_k * cos(pi*(2j+1)*k/1024), shape [512, 512] f32."""
    j = np.arange(S, dtype=np.float64)
    k = np.arange(S, dtype=np.float64)
    c = np.cos(np.pi * (2.0 * j[:, None] + 1.0) * k[None, :] / (2.0 * S))
    s = np.full(S, np.sqrt(2.0 / S), dtype=np.float64)
    s[0] /= np.sqrt(2.0)
    return (c * s[None, :]).astype(np.float32)


def _build():
    nc = bacc.Bacc("TRN2", target_bir_lowering=False, debug=False)
    # Image passed as two halves: rows 0:256 forward, rows 511:255 reversed
    # (host-side flip) so fold pairs m' <-> 511-m' are partition-aligned with
    # plain positive-stride DMAs.
    imga_d = nc.dram_tensor(
        "imga", [BPC, H, S], mybir.dt.float32r, kind="ExternalInput"
    ).ap()
    imgb_d = nc.dram_tensor(
        "imgb", [BPC, H, S], mybir.dt.float32r, kind="ExternalInput"
    ).ap()
    ct_d = nc.dram_tensor("ct", [S, S], mybir.dt.float32r, kind="ExternalInput").ap()
    ce_d = nc.dram_tensor("ce", [H, H], mybir.dt.float32r, kind="ExternalInput").ap()
    co_d = nc.dram_tensor("co", [H, H], mybir.dt.float32r, kind="ExternalInput").ap()
    out_d = nc.dram_tensor("out", [BPC, S, S], mybir.dt.float32, kind="ExternalOutput").ap()

    out_v = out_d.rearrange("b (t p) q -> b p t q", p=128)
    imga_v = imga_d.rearrange("b (t p) n -> b p t n", p=128)
    imgb_v = imgb_d.rearrange("b (t p) n -> b p t n", p=128)
    ct_v = ct_d.rearrange("(t p) k -> t p k", p=128)
    ce_v = ce_d.rearrange("(t p) k -> t p k", p=128)
    co_v = co_d.rearrange("(t p) k -> t p k", p=128)

    with tile.TileContext(nc) as tc:
        with (
            tc.tile_pool(name="const", bufs=1) as cpool,
            tc.tile_pool(name="a", bufs=10) as apool,
            tc.tile_pool(name="a01", bufs=8) as a01pool,
            tc.tile_pool(name="eo", bufs=16) as eopool,
            tc.tile_pool(name="dt", bufs=2) as dtpool,
            tc.tile_pool(name="o", bufs=8) as opool,
            tc.tile_pool(name="ps1", bufs=4, space="PSUM") as ps1pool,
            tc.tile_pool(name="ps2", bufs=4, space="PSUM") as ps2pool,
        ):
            # ce tile 0 first — the very first matmul needs only it plus
            # image 0's two halves; the remaining constants follow them.
            ce_sb = cpool.tile([128, 2, H], mybir.dt.float32r)
            co_sb = cpool.tile([128, 2, H], mybir.dt.float32r)
            ct_sb = cpool.tile([128, T, S], mybir.dt.float32r)
            nc.sync.dma_start(co_sb[:, 0, :], co_v[0])

            def emit_load_and_folds(i):
                """DMA image i's halves and emit the E/O folds; returns (e_t, o_t)."""
                if i < 2:
                    # Per-half loads in separate tiles: the first fold (and so
                    # the first matmul) starts after 512 KB instead of 1 MB.
                    ah = []
                    for mh, src in (
                        (0, imga_v[i, :, 0, :]),
                        (1, imgb_v[i, :, 0, :]),
                        (2, imga_v[i, :, 1, :]),
                        (3, imgb_v[i, :, 1, :]),
                    ):
                        t = a01pool.tile(
                            [128, S], mybir.dt.float32r, tag="a01", name=f"ah_{i}_{mh}"
                        )
                        nc.sync.dma_start(t[:], src)
                        ah.append(t)
                    af0, ar0, af1, ar1 = ah[0][:, :], ah[1][:, :], ah[2][:, :], ah[3][:, :]
                else:
                    af = apool.tile([128, 2, S], mybir.dt.float32r, tag="a", name=f"af_{i}")
                    ar = apool.tile([128, 2, S], mybir.dt.float32r, tag="a", name=f"ar_{i}")
                    nc.sync.dma_start(af[:], imga_v[i])
                    nc.sync.dma_start(ar[:], imgb_v[i])
                    af0, ar0, af1, ar1 = af[:, 0, :], ar[:, 0, :], af[:, 1, :], ar[:, 1, :]
                if i == 0:
                    # Remaining constants, ordered by first use.
                    nc.sync.dma_start(co_sb[:, 1, :], co_v[1])
                    for t in range(2):
                        nc.sync.dma_start(ce_sb[:, t, :], ce_v[t])
                    for t in range(T):
                        nc.sync.dma_start(ct_sb[:, t, :], ct_v[t])

                e0 = eopool.tile([128, S], mybir.dt.float32r, tag="eo", name=f"e0_{i}")
                e1 = eopool.tile([128, S], mybir.dt.float32r, tag="eo", name=f"e1_{i}")
                o0 = eopool.tile([128, S], mybir.dt.float32r, tag="eo", name=f"o0_{i}")
                o1 = eopool.tile([128, S], mybir.dt.float32r, tag="eo", name=f"o1_{i}")
                nc.gpsimd.tensor_sub(o0[:], af0, ar0)
                nc.gpsimd.tensor_sub(o1[:], af1, ar1)
                nc.vector.tensor_add(e0[:], af0, ar0)
                nc.vector.tensor_add(e1[:], af1, ar1)
                return (e0, e1), (o0, o1)

            folds = emit_load_and_folds(0)
            for i in range(BPC):
                e_t, o_t = folds

                # stage 1 (folded): Dt[n, 2k] from E/ce, Dt[n, 2k+1] from O/co.
                # ps1[nt] cols 0:256 hold even-p, cols 256:512 odd-p.
                ps1 = [ps1pool.tile([128, S], mybir.dt.float32, tag="ps1", name=f"ps1_{i}_{j}") for j in range(T)]
                # O-part first: the gpsimd subs are ready earlier than the DVE
                # adds (which queue behind the previous image's mid copies).
                for nt in range(T):
                    nts = slice(nt * 128, (nt + 1) * 128)
                    for mh in range(2):
                        nc.tensor.matmul(
                            ps1[nt][:, H:S],
                            o_t[mh][:, nts],
                            co_sb[:, mh, :],
                            start=(mh == 0),
                            stop=(mh == 1),
                        )
                    for mh in range(2):
                        nc.tensor.matmul(
                            ps1[nt][:, 0:H],
                            e_t[mh][:, nts],
                            ce_sb[:, mh, :],
                            start=(mh == 0),
                            stop=(mh == 1),
                        )
                # Prefetch the NEXT image's loads + folds now, so its DVE adds
                # run ahead of this image's mid copies in the DVE queue (the
                # folds were the once-per-image PE stall in the trace).
                if i + 1 < BPC:
                    folds = emit_load_and_folds(i + 1)

                dt_sb = dtpool.tile([128, T, S], mybir.dt.float32r, tag="dt")
                for nt in range(T):
                    # One mid-copy pair on ACT to keep DVE under the PE span.
                    eng = nc.scalar.copy if nt == 3 else nc.vector.tensor_copy
                    eng(dt_sb[:, nt, 0:S:2], ps1[nt][:, 0:H])
                    eng(dt_sb[:, nt, 1:S:2], ps1[nt][:, H:S])

                # stage 2 (p-outer): Y[p, q] = sum_n Dt[n, p] C'T[n, q]
                # Output staged in 2-tile chunks: fewer DMA descriptors while
                # keeping the drain pipelined.
                last = i == BPC - 1
                for ph in range(2):
                    o_sb = opool.tile(
                        [128, 2, S], mybir.dt.float32, tag="o", name=f"o_{i}_{ph}"
                    )
                    for pj in range(2):
                        pt = ph * 2 + pj
                        ps2 = ps2pool.tile(
                            [128, S], mybir.dt.float32, tag="ps2", name=f"ps2_{i}_{pt}"
                        )
                        for nt in range(T):
                            nc.tensor.matmul(
                                ps2[:],
                                dt_sb[:, nt, pt * 128 : (pt + 1) * 128],
                                ct_sb[:, nt, :],
                                start=(nt == 0),
                                stop=(nt == T - 1),
                            )
                        nc.scalar.copy(o_sb[:, pj, :], ps2[:])
                        if last:
                            # Drain the final image per p-tile on alternating
                            # queues so the tail DMA overlaps the last matmuls.
                            eng = nc.scalar if pt % 2 == 0 else nc.sync
                            eng.dma_start(out_v[i, :, pt, :], o_sb[:, pj, :])
                    if not last:
                        if ph == 0:
                            nc.scalar.dma_start(out_v[i, :, 0:2, :], o_sb[:])
                        else:
                            nc.sync.dma_start(out_v[i, :, 2:4, :], o_sb[:])
    nc.compile()
    return nc


_NC_CACHE = None


def _get_nc():
    global _NC_CACHE
    if _NC_CACHE is None:
        _NC_CACHE = _build()
    return _NC_CACHE


def run_sharded(img: np.ndarray, **spmd_kwargs):
    """img [128, 1, 512, 512] f32 -> (out [128, 1, 512, 512] f32, BassKernelResults)."""
    img = np.ascontiguousarray(np.asarray(img, dtype=np.float32)).reshape(B_FULL, S, S)
    imga = np.ascontiguousarray(img[:, :H, :])
    imgb = np.ascontiguousarray(img[:, :H - 1 :-1, :])  # rows 511..256 reversed
    ct = _basis_f32()
    ce = np.ascontiguousarray(ct[:H, 0::2])
    co = np.ascontiguousarray(ct[:H, 1::2])
    nc = _get_nc()
    in_maps = [
        {
            "imga": imga[k * BPC : (k + 1) * BPC],
            "imgb": imgb[k * BPC : (k + 1) * BPC],
            "ct": ct,
            "ce": ce,
            "co": co,
        }
        for k in range(N_CORES)
    ]
    res = run_bass_kernel_spmd(nc, in_maps, core_ids=list(range(N_CORES)), **spmd_kwargs)
    out = np.empty((B_FULL, S, S), dtype=np.float32)
    for k in range(N_CORES):
        out[k * BPC : (k + 1) * BPC] = res.results[k]["out"]
    return out.reshape(B_FULL, 1, S, S), res


def kernel(img: np.ndarray) -> np.ndarray:
    out, _ = run_sharded(img)
    return out
